# revision 19
# baseline (speedup 1.0000x reference)
"""AxialDCNv4 (dense_cnn) Trainium2 kernel — 8 NeuronCores.

Self-contained: kernel(**inputs) -> np.ndarray [2,128,160,160] f32.

Sharding: 8 cores = 2 batches x 4 H-bands of 40 rows; all conv weights
replicated.  Host<->device traffic over the axon tunnel (~60MB/s) is the
bottleneck, so per call we upload ONLY a disjoint fp16 channel-major band
per core ([C, 42*160], 1.7MB: own 40 rows + 1 conv halo row each side) plus
the small fp16 weights.  Each batch's full row-major fp16 image (and later
its full out_h) is assembled ON DEVICE with a 4-core AllGather into a
zero-padded [176*176, C] frame that the deformable gathers sample with
absolute per-core pixel bases — no out_h halo recompute, no row masks.
Device-resident constants (identity, kbias, pixel bases, output zero
buffers) are uploaded once and cached.  The output is downloaded as fp16.

Per-core pipeline (Bass/Tile):
  PE    : 50x transpose of own rows -> fp16 row-major staging in DRAM;
          fused (1x3) conv -> 90 offset+dyn channels [90, 40*160] (edge
          columns via partial-range PSUM accumulation, no column padding);
          fused (3x1) conv -> [90, 40*160]; per-128px-tile transposes.
  GPSIMD: AllGather [[0..3],[4..7]] of the row-major image / out_h;
          dma_gather (fp16 horizontal pixel-pairs, 512B descriptors) from
          the zero-padded full frames in DRAM (no masks/clamps needed).
  DVE   : positions/floor/fracs/bilinear corner weights, folded
          coeff[px, (j=36, g=8)] = w_corner * dynw, gather indices (int16),
          per-group TT-mult + segmented reduce over the 36 taps.
  agg-1 writes own out_h rows (fp16) -> AllGather -> padded frame; agg-2
  gathers from it and writes the final fp16 pixel-major band [6400, 128].
"""
import sys
import numpy as np

sys.path.insert(0, '/opt/trn_rl_repo')

import concourse.bass as bass
import concourse.mybir as mybir
import concourse.tile as tile_mod
from concourse.tile import TileContext
from concourse import library_config
from concourse.library_overlay import lower_extended_insts
from concourse.vector_clock import ScopedClock

# ---------------------------------------------------------------- patches --
# This walrus build cannot encode semaphore waits on Drain/NoOp CTRL
# instructions; Tile's final drain carries many.  Split them onto
# EventSemaphore instructions (<=2 waits each; we use 1).

def _patched_drain_and_barrier(self, tick_clock, wait_clock):
    nc = self.nc
    drain_inst = nc.sync.drain()
    wait_clock.add_sem_waits(
        drain_inst.ins, ScopedClock({None: tick_clock.global_clock})
    )
    si = drain_inst.ins.sync_info
    if si is not None and len(si.on_wait) > 0:
        waits = list(si.on_wait)
        si.on_wait.clear()
        rest = waits
        while rest:
            chunk, rest = rest[:1], rest[1:]
            nop = nc.sync.nop(nofuse=True, hint="drain_wait_split")
            nsi = nop.ins.sync_info
            if nsi is None:
                nop.ins.sync_info = mybir.SyncInfo(on_wait=list(chunk), on_update=[])
            else:
                nsi.on_wait.extend(chunk)
    nc.all_engine_barrier()
    assert self.sems is not None
    popped = nc._tile_sem_poison_stack.pop()
    assert popped is self._sem_poison
    nc.clear_and_free_semaphores(list(self.sems.allocated().values()))
    nc.all_engine_barrier()


tile_mod.TileContext._drain_and_barrier = _patched_drain_and_barrier


def split_waits(nc):
    """HW allows <=1 sync wait per instruction (EventSemaphore <=2)."""
    for fn in nc.m.functions:
        for bb in fn.blocks:
            insts = list(bb.instructions)
            out = []
            changed = False
            for inst in insts:
                si = inst.sync_info
                if si is not None and si.on_wait:
                    waits = list(si.on_wait)
                    cap = 2 if isinstance(inst, mybir.InstEventSemaphore) else 1
                    if len(waits) > cap:
                        si.on_wait.clear()
                        si.on_wait.extend(waits[:cap])
                        rest = waits[cap:]
                        while rest:
                            chunk, rest = rest[:2], rest[2:]
                            ev = mybir.InstEventSemaphore(
                                name=f"wsplit-{nc.next_id()}",
                                engine=inst.engine,
                                ins=[], outs=[],
                                sync_info=mybir.SyncInfo(on_wait=list(chunk),
                                                         on_update=[]),
                            )
                            nc.register_instruction(ev)
                            out.append(ev)
                            changed = True
                out.append(inst)
            if changed:
                bb.instructions.clear()
                bb.instructions.extend(out)


# ------------------------------------------------------------- constants --
H = W = 160
C = 128
K2 = 9
G = 8
OC = 90
PAD = 8
NBR = 40
NBW = NBR + 2          # uploaded band rows: r0-1 .. r0+40 (conv-v halo)
PW = W + 2 * PAD       # padded full-frame width
PH = H + 2 * PAD
NPIX_P = PH * PW
NPIX_B = NBR * W       # own-band pixels
NT = (NBR // 4) * (W // 32)   # 50 gather tiles per agg stage
MAGIC = 12582912.0

# names of external inputs that change per call (the rest are cached on
# device after the first call)
PER_CALL = ("x_cmh", "whT", "wvT", "bh", "bv")


def build_kernel():
    nc = bass.Bass("TRN2", num_devices=8)
    f32 = mybir.dt.float32
    f16 = mybir.dt.float16
    i16 = mybir.dt.int16
    AL = mybir.AluOpType

    x_cmh = nc.dram_tensor("x_cmh", [C, NBR * W], f16, kind="ExternalInput")
    whT = nc.dram_tensor("whT", [C, 3 * OC], f16, kind="ExternalInput")
    wvT = nc.dram_tensor("wvT", [C, 3 * OC], f16, kind="ExternalInput")
    bh = nc.dram_tensor("bh", [OC, 1], f32, kind="ExternalInput")
    bv = nc.dram_tensor("bv", [OC, 1], f32, kind="ExternalInput")
    iden = nc.dram_tensor("iden", [128, 128], f32, kind="ExternalInput")
    idenh = nc.dram_tensor("idenh", [128, 128], f16, kind="ExternalInput")
    kbias = nc.dram_tensor("kbias", [128, 18], f32, kind="ExternalInput")
    pixb = nc.dram_tensor("pixb", [128, NT], f32, kind="ExternalInput")
    emask = nc.dram_tensor("emask", [128, 8], f32, kind="ExternalInput")
    out = nc.dram_tensor("out", [C, NPIX_B], f16, kind="ExternalOutput")

    xstage = nc.dram_tensor("xstage", [NPIX_B, C], f16)
    xgath = nc.dram_tensor("xgath", [4 * NPIX_B, C], f16)
    x_full = nc.dram_tensor("x_full", [NPIX_P, C], f16)
    hstage = nc.dram_tensor("hstage", [NPIX_B, C], f16)
    hgath = nc.dram_tensor("hgath", [4 * NPIX_B, C], f16)
    h_full = nc.dram_tensor("h_full", [NPIX_P, C], f16)
    estage = nc.dram_tensor("estage", [C * 2 * W], f16)
    egath = nc.dram_tensor("egath", [4 * C * 2 * W], f16)
    idxstage = nc.dram_tensor("idxstage", [2 * NT * 18 * 128], i16)

    nc.gpsimd.load_library(library_config.mlp)
    nreg1024 = nc.gpsimd.to_reg(1024)
    nreg256 = nc.gpsimd.to_reg(256)

    RG = [[0, 1, 2, 3], [4, 5, 6, 7]]

    with TileContext(nc) as tc:
        with (
            tc.tile_pool(name="persist", bufs=1) as pp,
            tc.tile_pool(name="work", bufs=3) as wp,
            tc.tile_pool(name="gath", bufs=3) as gp,
            tc.tile_pool(name="psum", bufs=2, space="PSUM") as psp,
            tc.tile_pool(name="psum2", bufs=2, space="PSUM") as psp2,
            tc.tile_pool(name="psum3", bufs=2, space="PSUM") as psp3,
        ):
            # x frame [C, 42 rows x 160]: rows 1..40 = own band (uploaded),
            # rows 0 and 41 = conv-v halo rows, selected below from a tiny
            # channel-major edge-row AllGather using per-core mask constants.
            nc.sync.dma_start(
                estage[:],
                bass.AP(x_cmh, 0, [[NBR * W, C], [(NBR - 1) * W, 2], [1, W]]))
            nc.gpsimd.collective_compute(
                "AllGather", AL.bypass, replica_groups=[[0, 1, 2, 3], [4, 5, 6, 7]],
                ins=[estage[:]], outs=[egath[:]])
            x_sb = pp.tile([C, NBW * W], f16)
            nc.sync.dma_start(x_sb[:, W:W + NBR * W], x_cmh[:])
            e_sb = pp.tile([C, 8 * W], f16)
            nc.sync.dma_start(e_sb[:], bass.AP(egath, 0,
                                               [[2 * W, C], [C * 2 * W, 4], [1, 2 * W]]))
            em_sb = pp.tile([128, 8], f32)
            nc.sync.dma_start(em_sb[:], emask[:])
            whT_sb = pp.tile([C, 3 * OC], f16)
            nc.sync.dma_start(whT_sb[:], whT[:])
            wvT_sb = pp.tile([C, 3 * OC], f16)
            nc.sync.dma_start(wvT_sb[:], wvT[:])
            bh_sb = pp.tile([OC, 1], f32)
            nc.sync.dma_start(bh_sb[:], bh[:])
            bv_sb = pp.tile([OC, 1], f32)
            nc.sync.dma_start(bv_sb[:], bv[:])
            id_sb = pp.tile([128, 128], f32)
            nc.sync.dma_start(id_sb[:], iden[:])
            idh_sb = pp.tile([128, 128], f16)
            nc.sync.dma_start(idh_sb[:], idenh[:])
            kb_sb = pp.tile([128, 18], f32)
            nc.sync.dma_start(kb_sb[:], kbias[:])
            pb_sb = pp.tile([128, NT], f32)
            nc.sync.dma_start(pb_sb[:], pixb[:])

            fdh = pp.tile([OC, NPIX_B], f32)
            fdv = pp.tile([OC, NPIX_B], f32)

            # zero the padded full frames (borders; interiors rewritten)
            zt = pp.tile([128, NPIX_P // 2], f16)
            nc.vector.memset(zt[:], 0.0)
            half = NPIX_P // 2
            nc.sync.dma_start(x_full[0:half, :], zt[:])
            nc.sync.dma_start(x_full[half:2 * half, :], zt[:])
            nc.sync.dma_start(h_full[0:half, :], zt[:])
            nc.sync.dma_start(h_full[half:2 * half, :], zt[:])

            # halo rows: x_sb row 0 = bottom edge of band above, row 41 =
            # top edge of band below (zero at batch boundaries via mask)
            eh, eb = e_sb[:].tensor, e_sb[:].offset
            mh, mb = em_sb[:].tensor, em_sb[:].offset
            for row, edge_off, m_off in ((0, W, 0), (NBW - 1, 0, 4)):
                tmp_e = wp.tile([128, 4, W], f32, tag="tmpe")
                nc.vector.tensor_tensor(
                    out=tmp_e[:],
                    in0=bass.AP(eh, eb + edge_off, [e_sb[:].ap[0], [2 * W, 4], [1, W]]),
                    in1=bass.AP(mh, mb + m_off, [em_sb[:].ap[0], [1, 4], [0, W]]),
                    op=AL.mult)
                th_e, tb_e = tmp_e[:].tensor, tmp_e[:].offset
                with nc.allow_low_precision(reason="one-hot 4-way select"):
                    nc.vector.tensor_reduce(
                        x_sb[:, row * W:(row + 1) * W],
                        bass.AP(th_e, tb_e, [tmp_e[:].ap[0], [1, W], [W, 4]]),
                        axis=mybir.AxisListType.X, op=AL.add)

            # relayout own rows (band rows 1..40) to row-major staging
            for t in range(NPIX_B // 128):
                pst = psp3.tile([128, 128], f16, tag="rlps")
                nc.tensor.transpose(pst[:], x_sb[:, W + t * 128:W + (t + 1) * 128],
                                    idh_sb[:])
                st = wp.tile([128, 128], f16, tag="rlsb")
                nc.scalar.copy(st[:], pst[:])
                nc.sync.dma_start(xstage[t * 128:(t + 1) * 128, :], st[:])

            def gather_full(stage, gth, full):
                nc.gpsimd.collective_compute(
                    "AllGather", AL.bypass, replica_groups=RG,
                    ins=[stage[:]], outs=[gth[:]])
                dst = bass.AP(full, (PAD * PW + PAD) * C,
                              [[PW * C, H], [1, W * C]])
                nc.sync.dma_start(dst, gth[:])

            gather_full(xstage, xgath, x_full)

            x_v = x_sb[:].rearrange("c (r w) -> c r w", r=NBW)

            def conv_h(fd, wT_sb, b_sb):
                # (1,3) conv, zero col-pad via partial-range PSUM accumulate
                for r in range(NBR):
                    ps = psp.tile([OC, W], f32, tag="convps")
                    row = x_v[:, r + 1, :]
                    nc.tensor.matmul(ps[:], wT_sb[:, OC:2 * OC], row,
                                     start=True, stop=False)
                    nc.tensor.matmul(ps[:, 1:W], wT_sb[:, 0:OC],
                                     x_v[:, r + 1, 0:W - 1],
                                     start=False, stop=False, skip_group_check=True)
                    nc.tensor.matmul(ps[:, 0:W - 1], wT_sb[:, 2 * OC:3 * OC],
                                     x_v[:, r + 1, 1:W],
                                     start=False, stop=True, skip_group_check=True)
                    nc.scalar.activation(fd[:, r * W:(r + 1) * W], ps[:],
                                         mybir.ActivationFunctionType.Identity,
                                         bias=b_sb[:], scale=1.0)

            def conv_v(fd, wT_sb, b_sb):
                for r in range(NBR):
                    ps = psp.tile([OC, W], f32, tag="convps")
                    for t in range(3):
                        nc.tensor.matmul(ps[:], wT_sb[:, t * OC:(t + 1) * OC],
                                         x_v[:, r + t, :],
                                         start=(t == 0), stop=(t == 2))
                    nc.scalar.activation(fd[:, r * W:(r + 1) * W], ps[:],
                                         mybir.ActivationFunctionType.Identity,
                                         bias=b_sb[:], scale=1.0)

            conv_h(fdh, whT_sb, bh_sb)
            conv_v(fdv, wvT_sb, bv_sb)

            def agg(fd, src_rm, istage_base):
                for qi in range(NBR // 4):
                    for wj in range(5):
                        ti = qi * 5 + wj
                        chunk = bass.AP(fd[:].tensor,
                                        fd[:].offset + (qi * 4 * W + wj * 32),
                                        [fd[:].ap[0], [W, 4], [1, 32]])
                        chc = wp.tile([OC, 128], f32, tag="chc")
                        nc.scalar.copy(chc[:], chunk)
                        pst = psp2.tile([128, OC], f32, tag="tp")
                        nc.tensor.transpose(pst[:], chc[:], id_sb[:OC, :OC])
                        T = wp.tile([128, OC], f32, tag="T")
                        nc.scalar.copy(T[:], pst[:])
                        pos = wp.tile([128, 18], f32, tag="pos")
                        nc.vector.tensor_tensor(out=pos[:], in0=T[:, 0:18],
                                                in1=kb_sb[:], op=AL.add)
                        fl = wp.tile([128, 18], f32, tag="fl")
                        nc.vector.tensor_scalar(fl[:], pos[:], -0.5, MAGIC,
                                                AL.add, AL.add)
                        nc.vector.tensor_scalar_sub(fl[:], fl[:], MAGIC)
                        fr = wp.tile([128, 18], f32, tag="fr")
                        nc.vector.tensor_tensor(out=fr[:], in0=pos[:], in1=fl[:],
                                                op=AL.subtract)
                        om = wp.tile([128, 18], f32, tag="om")
                        nc.scalar.activation(om[:], fr[:],
                                             mybir.ActivationFunctionType.Identity,
                                             bias=1.0, scale=-1.0)
                        w4 = wp.tile([128, 36], f32, tag="w4")
                        omy, omx = om[:, 0:9], om[:, 9:18]
                        fy, fx = fr[:, 0:9], fr[:, 9:18]
                        w4h, base = w4[:].tensor, w4[:].offset

                        def w4s(off):
                            return bass.AP(w4h, base + off, [w4[:].ap[0], [4, 9]])
                        nc.vector.tensor_tensor(out=w4s(0), in0=omy, in1=omx, op=AL.mult)
                        nc.vector.tensor_tensor(out=w4s(1), in0=omy, in1=fx, op=AL.mult)
                        nc.vector.tensor_tensor(out=w4s(2), in0=fy, in1=omx, op=AL.mult)
                        nc.vector.tensor_tensor(out=w4s(3), in0=fy, in1=fx, op=AL.mult)
                        coef = wp.tile([128, 288], f32, tag="coef")
                        w4_e = bass.AP(w4h, base, [w4[:].ap[0], [4, 9], [1, 4], [0, 8]])
                        Th = T[:].tensor
                        dyn_e = bass.AP(Th, T[:].offset + 18,
                                        [T[:].ap[0], [1, 9], [0, 4], [9, 8]])
                        nc.vector.tensor_tensor(out=coef[:], in0=w4_e, in1=dyn_e,
                                                op=AL.mult)
                        y0, x0 = fl[:, 0:9], fl[:, 9:18]
                        idf = wp.tile([128, 18], f32, tag="idf")
                        ifh, ifb = idf[:].tensor, idf[:].offset
                        iftop = bass.AP(ifh, ifb, [idf[:].ap[0], [2, 9]])
                        ifbot = bass.AP(ifh, ifb + 1, [idf[:].ap[0], [2, 9]])
                        nc.vector.tensor_scalar_mul(iftop, y0, float(PW))
                        nc.vector.tensor_tensor(out=iftop, in0=iftop, in1=x0, op=AL.add)
                        nc.vector.tensor_scalar_add(iftop, iftop, pb_sb[:, ti:ti + 1])
                        nc.vector.tensor_scalar_add(ifbot, iftop, float(PW))
                        idi = wp.tile([128, 18], i16, tag="idi")
                        nc.vector.tensor_copy(idi[:], idf[:])
                        # store directly in wrapped DRAM layout:
                        # DRAM[q*144 + col*8 + L] = idi[L*16 + q, col]
                        sbase = istage_base + ti * 18 * 128
                        st_ap = bass.AP(idxstage, sbase, [[1, 8], [144, 16], [8, 18]])
                        nc.sync.dma_start(st_ap, idi[:])
                        wrap = wp.tile([128, 144], i16, tag="wrap")
                        ld_ap = bass.AP(idxstage, sbase, [[0, 8], [144, 16], [1, 144]])
                        nc.sync.dma_start(wrap[:], ld_ap)
                        gA = gp.tile([128, 18, 2, 128], f16, tag="gA")
                        src_ov = bass.AP(src_rm, 0, [[128, NPIX_P - 1], [1, 256]])
                        gAh, gAb = gA[:].tensor, gA[:].offset

                        def gsl(b0, nb):
                            return bass.AP(gAh, gAb + b0 * 256,
                                           [gA[:].ap[0], [256, nb], [1, 256]])
                        nc.gpsimd.dma_gather(gsl(0, 8), src_ov, wrap[:, 0:64],
                                             num_idxs=1024, num_idxs_reg=nreg1024,
                                             elem_size=256, elem_step=128)
                        nc.gpsimd.dma_gather(gsl(8, 8), src_ov, wrap[:, 64:128],
                                             num_idxs=1024, num_idxs_reg=nreg1024,
                                             elem_size=256, elem_step=128)
                        nc.gpsimd.dma_gather(gsl(16, 2), src_ov, wrap[:, 128:144],
                                             num_idxs=256, num_idxs_reg=nreg256,
                                             elem_size=256, elem_step=128)
                        of = wp.tile([128, 128], f32, tag="of")
                        tmp = wp.tile([128, 8, 576], f32, tag="tmp")
                        gh, gb = gA[:].tensor, gA[:].offset
                        ch, cb = coef[:].tensor, coef[:].offset
                        th, tb = tmp[:].tensor, tmp[:].offset
                        for g in range(G):
                            in0 = bass.AP(gh, gb + g * 16,
                                          [gA[:].ap[0], [256, 18], [128, 2], [1, 16]])
                            in1 = bass.AP(ch, cb + g,
                                          [coef[:].ap[0], [16, 18], [8, 2], [0, 16]])
                            nc.vector.tensor_tensor(out=tmp[:, g, :], in0=in0, in1=in1,
                                                    op=AL.mult)
                        red_in = bass.AP(th, tb, [tmp[:].ap[0], [576, 8], [1, 16], [16, 36]])
                        nc.vector.tensor_reduce(of[:], red_in,
                                                axis=mybir.AxisListType.X, op=AL.add)
                        yield ti, of

            for ti, of in agg(fdh, x_full, 0):
                qi, wj = ti // 5, ti % 5
                ob = wp.tile([128, 128], f16, tag="ob")
                nc.scalar.copy(ob[:], of[:])
                doff = ((qi * 4) * W + wj * 32) * C
                dst = bass.AP(hstage, doff, [[W * C, 4], [C, 32], [1, 128]])
                nc.sync.dma_start(dst, ob[:])

            gather_full(hstage, hgath, h_full)

            for ti, of in agg(fdv, h_full, NT * 18 * 128):
                qi, wj = ti // 5, ti % 5
                # transpose to channel-major so the host unshard is a
                # contiguous cast instead of a strided fp16 transpose
                pso = psp2.tile([128, 128], f32, tag="otp")
                nc.tensor.transpose(pso[:], of[:], id_sb[:])
                oh = wp.tile([128, 128], f16, tag="oh")
                nc.scalar.copy(oh[:], pso[:])
                doff = (qi * 4) * W + wj * 32
                dst = bass.AP(out, doff, [[NPIX_B, 128], [W, 4], [1, 32]])
                nc.sync.dma_start(dst, oh[:])

    lower_extended_insts(nc)
    split_waits(nc)
    return nc


# ------------------------------------------------------------- host side --

def prep_per_call(inputs):
    """Per-call inputs: x bands (fp16 ch-major, 42 rows) + fp16 weights."""
    x = np.asarray(inputs['x'])
    w_h = np.concatenate([inputs['w_hoff'], inputs['w_hw']], axis=0)
    w_v = np.concatenate([inputs['w_voff'], inputs['w_vw']], axis=0)
    b_h = np.concatenate([inputs['b_hoff'], inputs['b_hw']])[:, None].astype(np.float32)
    b_v = np.concatenate([inputs['b_voff'], inputs['b_vw']])[:, None].astype(np.float32)
    whT = np.ascontiguousarray(np.asarray(w_h)[:, :, 0, :].transpose(1, 2, 0)).reshape(C, 3 * OC).astype(np.float16)
    wvT = np.ascontiguousarray(np.asarray(w_v)[:, :, :, 0].transpose(1, 2, 0)).reshape(C, 3 * OC).astype(np.float16)

    # disjoint 40-row bands, fused f16 cast + band transpose in one pass
    xbands = np.empty((2, 4, C, NBR, W), np.float16)
    xbands[:] = x.reshape(2, C, 4, NBR, W).transpose(0, 2, 1, 3, 4)

    rep = lambda a: np.concatenate([a] * 8, axis=0)
    return {
        "x_cmh": xbands.reshape(8 * C, NBR * W),
        "whT": rep(whT), "wvT": rep(wvT), "bh": rep(b_h), "bv": rep(b_v),
    }


def make_consts():
    """Input-independent constants, uploaded once and cached on device."""
    ii = np.arange(K2) // 3
    jj = np.arange(K2) % 3
    kb = np.zeros((128, 18), np.float32)
    kb[:, 0:9] = (ii - 1)[None, :]
    kb[:, 9:18] = (jj - 1)[None, :]

    ri = np.arange(128) // 32
    wi = np.arange(128) % 32
    pixb = np.zeros((8, 128, NT), np.float32)
    for core in range(8):
        r0 = (core % 4) * NBR
        for ti in range(NT):
            qi, wj = ti // 5, ti % 5
            pixb[core, :, ti] = (r0 + qi * 4 + ri + PAD) * PW + wj * 32 + wi + PAD

    # emask[core]: cols 0..3 one-hot pick of the band above (up halo),
    # cols 4..7 of the band below (down halo); all-zero at batch edges
    emask = np.zeros((8, 128, 8), np.float32)
    for core in range(8):
        band = core % 4
        if band > 0:
            emask[core, :, band - 1] = 1.0
        if band < 3:
            emask[core, :, 4 + band + 1] = 1.0

    rep = lambda a: np.concatenate([a] * 8, axis=0)
    return {
        "iden": rep(np.eye(128, dtype=np.float32)),
        "idenh": rep(np.eye(128, dtype=np.float16)),
        "kbias": rep(kb),
        "pixb": pixb.reshape(8 * 128, NT),
        "emask": emask.reshape(8 * 128, 8),
    }


def unshard(out_concat):
    """out_concat: [8*C, NPIX_B] fp16 channel-major -> [2, C, H, W] f32."""
    o = np.asarray(out_concat).reshape(2, 4, C, NBR, W)
    full = np.empty((2, C, H, W), np.float32)
    full.reshape(2, C, 4, NBR, W)[:] = o.transpose(0, 2, 1, 3, 4)
    return full


# --------------------------------------------------------------- runner --

_CACHED = {}


def _make_runner(nc, n_cores=8):
    import jax
    from concourse import bass2jax
    from jax.sharding import Mesh, PartitionSpec, NamedSharding
    from jax.experimental.shard_map import shard_map

    bass2jax.install_neuronx_cc_hook()
    partition_name = nc.partition_id_tensor.name if nc.partition_id_tensor else None
    in_names, out_names, out_avals, zero_outs = [], [], [], []
    for alloc in nc.m.functions[0].allocations:
        if not isinstance(alloc, mybir.MemoryLocationSet):
            continue
        name = alloc.memorylocations[0].name
        if alloc.kind == "ExternalInput":
            if name != partition_name:
                in_names.append(name)
        elif alloc.kind == "ExternalOutput":
            shape = tuple(alloc.tensor_shape)
            dtype = mybir.dt.np(alloc.dtype)
            out_names.append(name)
            out_avals.append(jax.core.ShapedArray(shape, dtype))
            zero_outs.append(np.zeros((n_cores * shape[0], *shape[1:]), dtype))
    n_params = len(in_names)
    all_in = in_names + out_names + ([partition_name] if partition_name else [])

    def _body(*args):
        operands = list(args)
        if partition_name is not None:
            operands.append(bass2jax.partition_id_tensor())
        outs = bass2jax._bass_exec_p.bind(
            *operands, out_avals=tuple(out_avals), in_names=tuple(all_in),
            out_names=tuple(out_names), lowering_input_output_aliases=(),
            sim_require_finite=False, sim_require_nnan=False, nc=nc)
        return tuple(outs)

    devices = jax.devices()[:n_cores]
    mesh = Mesh(np.asarray(devices), ("core",))
    sharding = NamedSharding(mesh, PartitionSpec("core"))
    sharded = jax.jit(
        shard_map(_body, mesh=mesh,
                  in_specs=(PartitionSpec("core"),) * (n_params + len(out_avals)),
                  out_specs=(PartitionSpec("core"),) * len(out_avals),
                  check_rep=False),
        keep_unused=True)

    # device-resident constants + zero output buffers (uploaded once)
    consts = make_consts()
    dev_const = {k: jax.device_put(v, sharding) for k, v in consts.items()}
    dev_zeros = [jax.device_put(z, sharding) for z in zero_outs]
    jax.block_until_ready(list(dev_const.values()) + dev_zeros)

    def run(per_call):
        args = [per_call[name] if name in per_call else dev_const[name]
                for name in in_names]
        out_arrs = jax.block_until_ready(sharded(*args, *dev_zeros))
        return np.asarray(out_arrs[0])

    return run


def kernel(**inputs) -> np.ndarray:
    if "run" not in _CACHED:
        nc = build_kernel()
        _CACHED["run"] = _make_runner(nc)
    per_call = prep_per_call(inputs)
    out_concat = _CACHED["run"](per_call)
    return unshard(out_concat)


if __name__ == "__main__":
    rng = np.random.default_rng(0)
    demo = {
        'x': rng.standard_normal((2, C, H, W), dtype=np.float32),
        'w_hoff': rng.standard_normal((18, C, 1, 3), dtype=np.float32) * 0.05,
        'b_hoff': np.zeros(18, np.float32),
        'w_hw': rng.standard_normal((72, C, 1, 3), dtype=np.float32) * 0.05,
        'b_hw': np.zeros(72, np.float32),
        'w_voff': rng.standard_normal((18, C, 3, 1), dtype=np.float32) * 0.05,
        'b_voff': np.zeros(18, np.float32),
        'w_vw': rng.standard_normal((72, C, 3, 1), dtype=np.float32) * 0.05,
        'b_vw': np.zeros(72, np.float32),
    }
    out = kernel(**demo)
    print("kernel output", out.shape, out.dtype)
